# revision 1
# baseline (speedup 1.0000x reference)
"""BiLSTM (B=128, T=256, H=512, L=2) Trainium2 Bass kernel, v4.

Sharding: 8 cores = 2 directions x 4 batch-quarters (B_loc=32), data-parallel.
Each core runs 4 sequential phases: xproj0 GEMM -> layer-0 scan -> xproj1 GEMM
-> layer-1 scan. Host pre-flips time for backward cores and re-assembles.

Instruction-count-oriented choices:
 - float32r matmuls (self-loading: no separate Ldweights instruction)
 - gate columns host-reordered to [i, f, o, g]: one sigmoid over 1536 cols,
   one tanh over 512
 - h^T for the next step's matmuls produced by a DRAM round-trip with a
   transposing access pattern (2 DMAs) instead of 4 PE transposes + copy
 - batched DMAs (4 steps of xproj per load, 2 GEMM m-tiles per store)
"""

import numpy as np

import concourse.bacc as bacc
import concourse.mybir as mybir
import concourse.tile as tile
from concourse import bass_utils

F32 = mybir.dt.float32
F32R = mybir.dt.float32r
BF16 = mybir.dt.bfloat16
AF = mybir.ActivationFunctionType
OP = mybir.AluOpType

B_FULL, T_FULL, H, L = 128, 256, 512, 2
G = 4 * H          # 2048
KT = H // 128      # 4
NT = G // 512      # 4
NCORES = 8
B = B_FULL // 4    # 32 per core


def build_bilstm(T=T_FULL, reps=1):
    assert T % 8 == 0
    nc = bacc.Bacc("TRN2", target_bir_lowering=False, debug=False)

    xt_in = nc.dram_tensor("xt", [128, KT, T // 4, 128], F32R,
                           kind="ExternalInput").ap()
    wx0_in = nc.dram_tensor("wx0", [128, KT, G], F32R, kind="ExternalInput").ap()
    wh0_in = nc.dram_tensor("wh0", [128, KT, G], F32R, kind="ExternalInput").ap()
    wx1_in = nc.dram_tensor("wx1", [128, KT, G], F32R, kind="ExternalInput").ap()
    wh1_in = nc.dram_tensor("wh1", [128, KT, G], F32R, kind="ExternalInput").ap()
    b0_in = nc.dram_tensor("b0", [128, G], F32, kind="ExternalInput").ap()
    b1_in = nc.dram_tensor("b1", [128, G], F32, kind="ExternalInput").ap()
    # out[t, kt, p, b] = h1[b, t, kt*128+p]
    out = nc.dram_tensor("out", [T, KT, 128, B], F32R, kind="ExternalOutput").ap()

    with tile.TileContext(nc) as tc:
        with (
            tc.tile_pool(name="sb", bufs=2) as sb,
            tc.tile_pool(name="sb1", bufs=1) as sb1,
            tc.tile_pool(name="ps", bufs=2, space="PSUM") as psp,
            tc.tile_pool(name="dram", bufs=1, space="DRAM") as dram,
        ):
            def load_w(w_ap, b_ap):
                wsb = sb1.tile([128, KT, G], F32R, tag="wsb")
                nc.sync.dma_start(wsb[:], w_ap[:])
                if b_ap is None:
                    return wsb, None
                bsb = sb1.tile([128, G], F32, tag="bsb")
                nc.sync.dma_start(bsb[:], b_ap[:])
                return wsb, bsb

            def gemm(stat_of, wsb, bsb, xp_dst):
                """T//4 m-tiles (rows = 4 t-major timesteps x 32 batch)."""
                for mt in range(T // 4):
                    stat = stat_of(mt)     # kt -> lhsT [128, 128]
                    pg = psp.tile([128, G], F32, tag="pg")
                    for kt in range(KT):
                        for n in range(NT):
                            nc.tensor.matmul(
                                pg[:, n * 512:(n + 1) * 512],
                                stat(kt),
                                wsb[:, kt, n * 512:(n + 1) * 512],
                                start=(kt == 0),
                                stop=(kt == KT - 1),
                            )
                    if mt % 4 == 0:
                        xo4 = sb.tile([128, 4, G], BF16, tag="xo4")
                    nc.vector.tensor_tensor(
                        xo4[:, mt % 4, :], pg[:], bsb[:], op=OP.add
                    )
                    if mt % 4 == 3:
                        nc.sync.dma_start(
                            xp_dst[mt * 4 - 12:mt * 4 + 4, :, :].rearrange(
                                "(m t) b g -> (t b) m g", m=4
                            ),
                            xo4[:],
                        )

            def scan(wsb, xp_src, hbufT):
                """T LSTM steps reading xproj [T, 32, G], writing hbufT
                [128, KT, T, 32] (transposed h history)."""
                prev_c = None
                for t in range(T):
                    first = t == 0
                    if t % 8 == 0:
                        xp8 = sb1.tile([32, 8, G], BF16, tag="xp8")
                        nc.sync.dma_start(
                            xp8[:],
                            xp_src[t:t + 8, :, :].rearrange("t b g -> b t g"),
                        )
                    xp = xp8[:, t % 8, :]

                    if first:
                        ga_src = xp
                    else:
                        pg = psp.tile([B, G], F32, tag="pg")
                        for kt in range(KT):
                            for n in range(NT):
                                nc.tensor.matmul(
                                    pg[:, n * 512:(n + 1) * 512],
                                    hT[:, kt, :],
                                    wsb[:, kt, n * 512:(n + 1) * 512],
                                    start=(kt == 0),
                                    stop=(kt == KT - 1),
                                )
                        ga = sb1.tile([32, G], BF16, tag="ga")
                        nc.vector.tensor_tensor(ga[:], pg[:], xp, op=OP.add)
                        ga_src = ga[:]

                    gact = sb1.tile([32, G], F32, tag="gact")
                    nc.scalar.activation(gact[:], ga_src[:], AF.Sigmoid)
                    # cols: i 0:512 | f 512:1024 | o 1024:1536 | g 1536:2048
                    c_new = sb.tile([32, H], F32, tag="c_st")
                    if first:
                        m1h = sb1.tile([32, H], F32, tag="m1")
                        nc.vector.scalar_tensor_tensor(
                            m1h[:], gact[:, 1536:2048], 0.5, gact[:, 0:512],
                            op0=OP.subtract, op1=OP.mult,
                        )
                        nc.vector.tensor_scalar_mul(c_new[:], m1h[:], 2.0)
                    else:
                        m2 = sb1.tile([32, H], F32, tag="m2")
                        nc.vector.tensor_tensor(
                            m2[:], gact[:, 512:1024], prev_c[:], op=OP.mult
                        )
                        m1h = sb1.tile([32, H], F32, tag="m1")
                        nc.vector.scalar_tensor_tensor(
                            m1h[:], gact[:, 1536:2048], 0.5, gact[:, 0:512],
                            op0=OP.subtract, op1=OP.mult,
                        )
                        nc.vector.scalar_tensor_tensor(
                            c_new[:], m1h[:], 2.0, m2[:],
                            op0=OP.mult, op1=OP.add,
                        )
                    tch = sb1.tile([32, H], F32, tag="tch")
                    nc.scalar.activation(tch[:], c_new[:], AF.Tanh)
                    h = sb1.tile([32, KT, 128], F32R, tag="h")
                    nc.vector.tensor_tensor(
                        h[:].rearrange("b kt p -> b (kt p)"),
                        gact[:, 1024:1536], tch[:], op=OP.mult,
                    )
                    # h -> DRAM, then load back transposed as hT
                    nc.sync.dma_start(
                        hbufT[t, :, :, :].rearrange("kt p b -> b kt p"), h[:]
                    )
                    hT = sb.tile([128, KT, B], F32R, tag="hT")
                    nc.sync.dma_start(
                        hT[:],
                        hbufT[t, :, :, :].rearrange("kt p b -> p kt b"),
                    )
                    prev_c = c_new

            for _rep in range(reps):
                xproj0 = dram.tile([T, B, G], BF16, tag="xproj0")
                xproj1 = dram.tile([T, B, G], BF16, tag="xproj1")
                hbufT0 = dram.tile([T, KT, 128, B], F32R, tag="hbufT0")

                # P1: xproj0 = x @ wx0 + b0
                wsb, bsb = load_w(wx0_in, b0_in)
                cur = {}

                def stat_x(mt):
                    if mt % 4 == 0:
                        xt4 = sb.tile([128, KT, 4, 128], F32R, tag="xt4")
                        nc.sync.dma_start(
                            xt4[:], xt_in[:, :, mt:mt + 4, :]
                        )
                        cur["xt4"] = xt4
                    return lambda kt, m=mt % 4: cur["xt4"][:, kt, m, :]

                gemm(stat_x, wsb, bsb, xproj0[:])

                # P2: layer-0 scan
                wsb, _ = load_w(wh0_in, None)
                scan(wsb, xproj0[:], hbufT0[:])

                # P3: xproj1 = h0 @ wx1 + b1
                wsb, bsb = load_w(wx1_in, b1_in)
                cur1 = {}

                def stat_h(mt):
                    if mt % 4 == 0:
                        hT16 = sb.tile([128, KT, 16, B], F32R, tag="hT16")
                        for kt in range(KT):
                            nc.sync.dma_start(
                                hT16[:, kt, :, :],
                                hbufT0[mt * 4:mt * 4 + 16, kt, :, :].rearrange(
                                    "t p b -> p t b"
                                ),
                            )
                        cur1["hT16"] = hT16
                    return lambda kt, m=mt % 4: cur1["hT16"][
                        :, kt, m * 4:(m + 1) * 4, :
                    ].rearrange("p t b -> p (t b)")

                gemm(stat_h, wsb, bsb, xproj1[:])

                # P4: layer-1 scan -> out
                wsb, _ = load_w(wh1_in, None)
                scan(wsb, xproj1[:], out)

    nc.compile()
    return nc


_NC_CACHE = {}


def _get_nc(T=T_FULL):
    if T not in _NC_CACHE:
        _NC_CACHE[T] = build_bilstm(T=T)
    return _NC_CACHE[T]


def _perm_cols(a):
    """gate columns [i f g o] -> [i f o g] along last axis (size G)."""
    return np.concatenate(
        [a[..., 0:512], a[..., 512:1024], a[..., 1536:2048], a[..., 1024:1536]],
        axis=-1,
    )


def _pack_w(w):
    w = _perm_cols(w).copy()
    w[..., 1536:2048] *= 2.0
    w = w.reshape(KT, 128, G).transpose(1, 0, 2)
    return np.ascontiguousarray(w, dtype=np.float32)


def _pack_bias(b):
    b = _perm_cols(b).copy()
    b[..., 1536:2048] *= 2.0
    return np.ascontiguousarray(
        np.broadcast_to(b, (128, G)), dtype=np.float32
    )


def _pack_xt(x, T):
    """[B, T, H] -> [128, KT, T//4, 128] : [p, kt, mt, (4t x 32b) t-major]."""
    xt = x.transpose(2, 1, 0)                       # [H, T, B]
    xt = xt.reshape(KT, 128, T // 4, 4, B)
    xt = xt.transpose(1, 0, 2, 3, 4).reshape(128, KT, T // 4, 128)
    return np.ascontiguousarray(xt, dtype=np.float32)


def _shard_inputs(x, Wx, Wh, b):
    T = x.shape[1]
    in_maps = []
    packed = {}
    for d in range(2):
        packed[d] = {
            "wx0": _pack_w(Wx[0, d]), "wh0": _pack_w(Wh[0, d]),
            "wx1": _pack_w(Wx[1, d]), "wh1": _pack_w(Wh[1, d]),
            "b0": _pack_bias(b[0, d]), "b1": _pack_bias(b[1, d]),
        }
    for r in range(NCORES):
        d, q = r // 4, r % 4
        xc = x[q * B:(q + 1) * B]
        if d == 1:
            xc = xc[:, ::-1, :]
        m = dict(packed[d])
        m["xt"] = _pack_xt(xc, T)
        in_maps.append(m)
    return in_maps


def _assemble(results, T=T_FULL):
    full = np.empty((B_FULL, T, 2 * H), dtype=np.float32)
    for r in range(NCORES):
        d, q = r // 4, r % 4
        o = np.asarray(results[r]["out"], dtype=np.float32)  # [T, KT, 128, B]
        o = o.transpose(3, 0, 1, 2).reshape(B, T, H)
        if d == 1:
            o = o[:, ::-1, :]
        full[q * B:(q + 1) * B, :, d * H:(d + 1) * H] = o
    return full


def run_kernel(x, Wx, Wh, b, trace=False):
    nc = _get_nc()
    in_maps = _shard_inputs(
        np.asarray(x), np.asarray(Wx), np.asarray(Wh), np.asarray(b)
    )
    res = bass_utils.run_bass_kernel_spmd(
        nc, in_maps, core_ids=list(range(NCORES)), trace=trace
    )
    return _assemble(res.results), res


def kernel(x, Wx, Wh, b):
    out, _ = run_kernel(x, Wx, Wh, b)
    return out



# revision 4
# speedup vs baseline: 240.5367x; 240.5367x over previous
"""BiLSTM (B=128, T=256, H=512, L=2) Trainium2 Bass kernel, v5.

Sharding: 8 cores = 2 directions x 4 batch-quarters (B_loc=32), data-parallel.
Each core runs 4 phases: xproj0 GEMM -> layer-0 scan -> xproj1 GEMM ->
layer-1 scan.  Host pre-flips time for backward cores and re-assembles.

v5 changes vs v4 (true device time ~47 ms):
 - h^T for the next step's recurrent matmul is produced ON-CHIP via 4 PE
   transposes + one DVE copy (v4 did a DRAM round-trip: 2 serialized DMAs
   on the critical path of every step).
 - gate blocks host-reordered to [g(x2), i, f, o] and the matmuls issued
   n-major with per-block accumulation groups, so sigmoid/c-update work for
   early blocks overlaps the later blocks' matmuls; the o block is the only
   one on the post-matmul critical path.
 - the o block's xproj term is accumulated INTO PSUM by an extra identity
   matmul (E32.T @ xp_o), so the tail is sigmoid(PSUM) -> h -> transpose
   with no DVE add.
 - all per-step tiles double-buffered (v4 used bufs=1 for most, serializing
   consecutive steps); xproj prefetched 4 steps at a time.
"""

import numpy as np

import concourse.bacc as bacc
import concourse.mybir as mybir
import concourse.tile as tile
from concourse import bass_utils

F32 = mybir.dt.float32
F32R = mybir.dt.float32r
BF16 = mybir.dt.bfloat16
AF = mybir.ActivationFunctionType
OP = mybir.AluOpType

B_FULL, T_FULL, H, L = 128, 256, 512, 2
G = 4 * H          # 2048
KT = H // 128      # 4
NT = G // 512      # 4
NCORES = 8
B = B_FULL // 4    # 32 per core


def build_bilstm(T=T_FULL, reps=1):
    assert T % 4 == 0
    nc = bacc.Bacc("TRN2", target_bir_lowering=False, debug=False)

    xt_in = nc.dram_tensor("xt", [128, KT, T // 4, 128], F32R,
                           kind="ExternalInput").ap()
    wx0_in = nc.dram_tensor("wx0", [128, KT, G], F32R, kind="ExternalInput").ap()
    wh0_in = nc.dram_tensor("wh0", [128, KT, G], F32R, kind="ExternalInput").ap()
    wx1_in = nc.dram_tensor("wx1", [128, KT, G], F32R, kind="ExternalInput").ap()
    wh1_in = nc.dram_tensor("wh1", [128, KT, G], F32R, kind="ExternalInput").ap()
    b0_in = nc.dram_tensor("b0", [128, G], F32, kind="ExternalInput").ap()
    b1_in = nc.dram_tensor("b1", [128, G], F32, kind="ExternalInput").ap()
    e32b_in = nc.dram_tensor("e32b", [32, 32], BF16, kind="ExternalInput").ap()
    e32r_in = nc.dram_tensor("e32r", [32, 32], F32R, kind="ExternalInput").ap()
    # out[t, kt, p, b] = h1[b, t, kt*128+p]
    out = nc.dram_tensor("out", [T, KT, 128, B], F32R, kind="ExternalOutput").ap()

    with tile.TileContext(nc) as tc:
        with (
            tc.tile_pool(name="sb", bufs=2) as sb,
            tc.tile_pool(name="sb1", bufs=1) as sb1,
            tc.tile_pool(name="ps", bufs=2, space="PSUM") as psp,
            tc.tile_pool(name="dram", bufs=1, space="DRAM") as dram,
        ):
            e32b = sb1.tile([32, 32], BF16, tag="e32b")
            nc.sync.dma_start(e32b[:], e32b_in[:])
            e32r = sb1.tile([32, 32], F32R, tag="e32r")
            nc.sync.dma_start(e32r[:], e32r_in[:])

            def load_w(w_ap, b_ap):
                wsb = sb1.tile([128, KT, G], F32R, tag="wsb")
                nc.sync.dma_start(wsb[:], w_ap[:])
                if b_ap is None:
                    return wsb, None
                bsb = sb1.tile([128, G], F32, tag="bsb")
                nc.sync.dma_start(bsb[:], b_ap[:])
                return wsb, bsb

            def gemm(stat_of, wsb, bsb, xp_dst):
                """T//4 m-tiles (rows = 4 t-major timesteps x 32 batch)."""
                for mt in range(T // 4):
                    stat = stat_of(mt)     # kt -> lhsT [128, 128]
                    pg = psp.tile([128, G], F32, tag="pg")
                    for kt in range(KT):
                        for n in range(NT):
                            nc.tensor.matmul(
                                pg[:, n * 512:(n + 1) * 512],
                                stat(kt),
                                wsb[:, kt, n * 512:(n + 1) * 512],
                                start=(kt == 0),
                                stop=(kt == KT - 1),
                            )
                    if mt % 4 == 0:
                        xo4 = sb.tile([128, 4, G], BF16, tag="xo4")
                    nc.vector.tensor_tensor(
                        xo4[:, mt % 4, :], pg[:], bsb[:], op=OP.add
                    )
                    if mt % 4 == 3:
                        nc.sync.dma_start(
                            xp_dst[mt * 4 - 12:mt * 4 + 4, :, :].rearrange(
                                "(m t) b g -> (t b) m g", m=4
                            ),
                            xo4[:],
                        )

            def scan(wsb, xp_src, hT_dst):
                """T LSTM steps reading xproj [T, 32, G] (bf16), writing
                transposed h history hT_dst [T, KT, 128, 32] (f32r).

                Gate blocks: 0=g (cols pre-scaled x2), 1=i, 2=f, 3=o.
                tanh(g) == 2*sigmoid(2g) - 1.
                """
                prev_c = None
                prev_hT = None
                for t in range(T):
                    first = t == 0
                    if t % 4 == 0:
                        xp4 = sb.tile([32, 4, G], BF16, tag="xp4")
                        nc.sync.dma_start(
                            xp4[:],
                            xp_src[t:t + 4, :, :].rearrange("t b g -> b t g"),
                        )
                    xp = xp4[:, t % 4, :]

                    gact = sb.tile([32, G], BF16, tag="gact")
                    if first:
                        nc.scalar.activation(gact[:], xp[:], AF.Sigmoid)
                    else:
                        pg = psp.tile([32, G], F32, tag="pg")
                        for n in range(NT):
                            for kt in range(KT):
                                nc.tensor.matmul(
                                    pg[:, n * 512:(n + 1) * 512],
                                    prev_hT[:, kt, :],
                                    wsb[:, kt, n * 512:(n + 1) * 512],
                                    start=(kt == 0),
                                    stop=(kt == KT - 1 and n < NT - 1),
                                )
                            if n < NT - 1:
                                # g/i/f: xp added on DVE, sigmoid from SBUF
                                ga = sb.tile([32, G], BF16, tag="ga")
                                nc.vector.tensor_tensor(
                                    ga[:, n * 512:(n + 1) * 512],
                                    pg[:, n * 512:(n + 1) * 512],
                                    xp[:, n * 512:(n + 1) * 512],
                                    op=OP.add,
                                )
                                nc.scalar.activation(
                                    gact[:, n * 512:(n + 1) * 512],
                                    ga[:, n * 512:(n + 1) * 512],
                                    AF.Sigmoid,
                                )
                        # o: xp folded into PSUM via identity matmul, so the
                        # tail is a single PSUM sigmoid.
                        nc.tensor.matmul(
                            pg[:, 3 * 512:], e32b[:], xp[:, 3 * 512:],
                            start=False, stop=True,
                        )
                        nc.scalar.activation(
                            gact[:, 3 * 512:], pg[:, 3 * 512:], AF.Sigmoid
                        )

                    # c_new = 2*(sig_g - 0.5)*sig_i + sig_f*c_prev
                    m1h = sb.tile([32, H], BF16, tag="m1h")
                    nc.vector.scalar_tensor_tensor(
                        m1h[:], gact[:, 0:512], 0.5, gact[:, 512:1024],
                        op0=OP.subtract, op1=OP.mult,
                    )
                    c_new = sb.tile([32, H], F32, tag="c_st")
                    if first:
                        nc.vector.tensor_scalar_mul(c_new[:], m1h[:], 2.0)
                    else:
                        fc = sb.tile([32, H], F32, tag="fc")
                        nc.vector.tensor_tensor(
                            fc[:], gact[:, 1024:1536], prev_c[:], op=OP.mult
                        )
                        nc.vector.scalar_tensor_tensor(
                            c_new[:], m1h[:], 2.0, fc[:],
                            op0=OP.mult, op1=OP.add,
                        )
                    tch = sb.tile([32, H], BF16, tag="tch")
                    nc.scalar.activation(tch[:], c_new[:], AF.Tanh)
                    hsb = sb.tile([32, KT, 128], F32R, tag="hsb")
                    nc.vector.tensor_tensor(
                        hsb[:].rearrange("b kt p -> b (kt p)"),
                        gact[:, 1536:2048], tch[:], op=OP.mult,
                    )
                    # hT via PE transpose (PSUM) + one DVE copy back to SBUF
                    pt = psp.tile([128, KT, B], F32R, tag="pg")
                    for c in range(KT):
                        nc.tensor.transpose(
                            pt[:, c, :], hsb[:, c, :], e32r[:]
                        )
                    hT = sb.tile([128, KT, B], F32R, tag="hT")
                    nc.vector.tensor_copy(hT[:], pt[:])
                    nc.sync.dma_start(
                        hT_dst[t, :, :, :].rearrange("kt p b -> p kt b"),
                        hT[:],
                    )
                    prev_c = c_new
                    prev_hT = hT

            for _rep in range(reps):
                xproj0 = dram.tile([T, B, G], BF16, tag="xproj0")
                xproj1 = dram.tile([T, B, G], BF16, tag="xproj1")
                hbufT0 = dram.tile([T, KT, 128, B], F32R, tag="hbufT0")

                # P1: xproj0 = x @ wx0 + b0
                wsb, bsb = load_w(wx0_in, b0_in)
                cur = {}

                def stat_x(mt):
                    if mt % 4 == 0:
                        xt4 = sb.tile([128, KT, 4, 128], F32R, tag="xt4")
                        nc.sync.dma_start(
                            xt4[:], xt_in[:, :, mt:mt + 4, :]
                        )
                        cur["xt4"] = xt4
                    return lambda kt, m=mt % 4: cur["xt4"][:, kt, m, :]

                gemm(stat_x, wsb, bsb, xproj0[:])

                # P2: layer-0 scan
                wsb, _ = load_w(wh0_in, None)
                scan(wsb, xproj0[:], hbufT0[:])

                # P3: xproj1 = h0 @ wx1 + b1
                wsb, bsb = load_w(wx1_in, b1_in)
                cur1 = {}

                def stat_h(mt):
                    if mt % 4 == 0:
                        hT16 = sb.tile([128, KT, 16, B], F32R, tag="hT16")
                        for kt in range(KT):
                            nc.sync.dma_start(
                                hT16[:, kt, :, :],
                                hbufT0[mt * 4:mt * 4 + 16, kt, :, :].rearrange(
                                    "t p b -> p t b"
                                ),
                            )
                        cur1["hT16"] = hT16
                    return lambda kt, m=mt % 4: cur1["hT16"][
                        :, kt, m * 4:(m + 1) * 4, :
                    ].rearrange("p t b -> p (t b)")

                gemm(stat_h, wsb, bsb, xproj1[:])

                # P4: layer-1 scan -> out
                wsb, _ = load_w(wh1_in, None)
                scan(wsb, xproj1[:], out)

    nc.compile()
    return nc


_NC_CACHE = {}


def _get_nc(T=T_FULL):
    if T not in _NC_CACHE:
        _NC_CACHE[T] = build_bilstm(T=T)
    return _NC_CACHE[T]


def _perm_cols(a):
    """gate columns [i f g o] -> [g i f o] along last axis (size G)."""
    return np.concatenate(
        [a[..., 1024:1536], a[..., 0:512], a[..., 512:1024], a[..., 1536:2048]],
        axis=-1,
    )


def _pack_w(w):
    w = _perm_cols(w).copy()
    w[..., 0:512] *= 2.0
    w = w.reshape(KT, 128, G).transpose(1, 0, 2)
    return np.ascontiguousarray(w, dtype=np.float32)


def _pack_bias(b):
    b = _perm_cols(b).copy()
    b[..., 0:512] *= 2.0
    return np.ascontiguousarray(
        np.broadcast_to(b, (128, G)), dtype=np.float32
    )


def _pack_xt(x, T):
    """[B, T, H] -> [128, KT, T//4, 128] : [p, kt, mt, (4t x 32b) t-major]."""
    xt = x.transpose(2, 1, 0)                       # [H, T, B]
    xt = xt.reshape(KT, 128, T // 4, 4, B)
    xt = xt.transpose(1, 0, 2, 3, 4).reshape(128, KT, T // 4, 128)
    return np.ascontiguousarray(xt, dtype=np.float32)


def _shard_inputs(x, Wx, Wh, b):
    import ml_dtypes

    T = x.shape[1]
    in_maps = []
    packed = {}
    eye = np.eye(32, dtype=np.float32)
    eye_bf = eye.astype(ml_dtypes.bfloat16)
    for d in range(2):
        packed[d] = {
            "wx0": _pack_w(Wx[0, d]), "wh0": _pack_w(Wh[0, d]),
            "wx1": _pack_w(Wx[1, d]), "wh1": _pack_w(Wh[1, d]),
            "b0": _pack_bias(b[0, d]), "b1": _pack_bias(b[1, d]),
        }
    for r in range(NCORES):
        d, q = r // 4, r % 4
        xc = x[q * B:(q + 1) * B]
        if d == 1:
            xc = xc[:, ::-1, :]
        m = dict(packed[d])
        m["xt"] = _pack_xt(xc, T)
        m["e32b"] = eye_bf
        m["e32r"] = eye
        in_maps.append(m)
    return in_maps


def _assemble(results, T=T_FULL):
    full = np.empty((B_FULL, T, 2 * H), dtype=np.float32)
    for r in range(NCORES):
        d, q = r // 4, r % 4
        o = np.asarray(results[r]["out"], dtype=np.float32)  # [T, KT, 128, B]
        o = o.transpose(3, 0, 1, 2).reshape(B, T, H)
        if d == 1:
            o = o[:, ::-1, :]
        full[q * B:(q + 1) * B, :, d * H:(d + 1) * H] = o
    return full


def run_kernel(x, Wx, Wh, b, trace=False):
    nc = _get_nc()
    in_maps = _shard_inputs(
        np.asarray(x), np.asarray(Wx), np.asarray(Wh), np.asarray(b)
    )
    res = bass_utils.run_bass_kernel_spmd(
        nc, in_maps, core_ids=list(range(NCORES)), trace=trace
    )
    return _assemble(res.results), res


def kernel(x, Wx, Wh, b):
    out, _ = run_kernel(x, Wx, Wh, b)
    return out
